# revision 43
# baseline (speedup 1.0000x reference)
"""Multi-head attention (B=2,S=2048,D=1024,H=16,A=64) on 8 trn2 NeuronCores.

Sharding: core = 4*b + g  (b = batch, g = head-group of 4 heads).
Per core, feature-on-partition layout throughout:
  qT,kT = matmuls of Wq/Wk vs xT;  v natural; S^T per head; softmax over keys
  via exp (no max-sub; scores ~ N(0,1)) with the denominator produced by a
  ones-column appended to v; normalized attT [C=256, S] per core.
  fc_out: each core computes its partial over the full sequence of its batch;
  the host sums the 4 partials per batch.

Schedule: software-pipelined units u=(pair, qc). Per block, AV+normalize of
unit i-1 overlaps the exp stream of unit i on ACT; projection work (v, qk of
tile 1) and fc_out chunks fill PE slack inside the ACT-bound stretches.
Softmax normalize = reciprocal_approx_fast (DVE) + partition_broadcast
(GpSimd) + one DVE multiply; per-qc fc_out spreads the output DMA.
"""

from collections import deque

import numpy as np

B, S, D, H, A = 2, 2048, 1024, 16, 64
GROUPS = 4              # head groups (cores per batch)
HPG = H // GROUPS       # heads per core = 4
C = HPG * A             # channels per core = 256
N_CORES = 8
SQ = S // GROUPS


def build_nc(s=S, d=D, n_cores=N_CORES):
    import concourse.bass as bass
    import concourse.mybir as mybir
    import concourse.tile as tile
    from concourse import bacc

    f32 = mybir.dt.float32
    bf16 = mybir.dt.bfloat16
    AF = mybir.ActivationFunctionType

    KD = d // 128        # d-tiles (contraction for projections) = 8
    MC = C // 128        # c-tiles per core = 2 (pairs of heads)
    NS = s // 128        # seq tiles = 16
    QC = s // 512        # q chunks = 4
    QW = 512             # q chunk width
    KT_PER_ST = 2        # k-tiles packed per st/pt tile (exp batching)
    NG = NS // KT_PER_ST # st/pt groups per (p, qc) = 8
    OW = 512             # output free-dim chunk

    nc = bacc.Bacc(
        "TRN2", target_bir_lowering=False, debug=False,
        enable_asserts=True, num_devices=n_cores,
    )

    xT_d = nc.dram_tensor("xT", [d, s], bf16, kind="ExternalInput").ap()
    wq_d = nc.dram_tensor("wq", [d, C], bf16, kind="ExternalInput").ap()
    wk_d = nc.dram_tensor("wk", [d, C], bf16, kind="ExternalInput").ap()
    wv_d = nc.dram_tensor("wv", [d, C], bf16, kind="ExternalInput").ap()
    wo_d = nc.dram_tensor("wo", [C, d], bf16, kind="ExternalInput").ap()
    bqs_d = nc.dram_tensor("bqs", [128, MC], f32, kind="ExternalInput").ap()
    bks_d = nc.dram_tensor("bks", [128, MC], f32, kind="ExternalInput").ap()
    bvb_d = nc.dram_tensor("bvb", [128, C], f32, kind="ExternalInput").ap()
    bob_d = nc.dram_tensor("bob", [128, d], f32, kind="ExternalInput").ap()
    # bf16 output partials (summed in f32 on host) — halves the output DMA
    out_d = nc.dram_tensor("out", [s, d], bf16, kind="ExternalOutput").ap()

    with tile.TileContext(nc) as tc:
        with tc.tile_pool(name="const", bufs=1) as cpool, \
             tc.tile_pool(name="qkv", bufs=1) as qpool, \
             tc.tile_pool(name="wop", bufs=1) as wopool, \
             tc.tile_pool(name="xTw", bufs=1) as xpool, \
             tc.tile_pool(name="ptp", bufs=10) as ptpool, \
             tc.tile_pool(name="sml", bufs=3) as spool, \
             tc.tile_pool(name="osb", bufs=3) as opool, \
             tc.tile_pool(name="ps1", bufs=2, space="PSUM") as pp1, \
             tc.tile_pool(name="pst", bufs=1, space="PSUM") as stp, \
             tc.tile_pool(name="pav", bufs=1, space="PSUM") as avp:

            # warm the ACT exp table set while input DMAs are in flight
            warm_f = cpool.tile([1, 16], f32, name="warm_f")
            nc.vector.memset(warm_f[:], 0.0)
            warm_o = cpool.tile([1, 16], f32, name="warm_o")
            nc.scalar.activation(warm_o[:], warm_f[:], AF.Exp, scale=1.0)

            bq_sb = cpool.tile([128, MC], f32, name="bq_sb")
            bk_sb = cpool.tile([128, MC], f32, name="bk_sb")
            bvb_sb = cpool.tile([128, C], f32, name="bvb_sb")
            bob_sb = cpool.tile([128, d], f32, name="bob_sb")

            # qT zero-padded per head-half: qTz[p][hh] holds head 2p+hh in
            # rows hh*A..hh*A+A, zeros elsewhere. Scores then contract the
            # full 128 rows of kT (zeros annihilate the other head), keeping
            # the PE in uniform 128x128 mode — no row-tiling mode switches.
            qTz_sb = [[qpool.tile([128, s], bf16, name=f"qTz{mt}_{hh}",
                                  tag=f"qTz{mt}_{hh}") for hh in range(2)]
                      for mt in range(MC)]
            for mt in range(MC):
                for hh in range(2):
                    nc.vector.memset(
                        qTz_sb[mt][hh][(1 - hh) * A:(2 - hh) * A, :], 0.0)
            kT_sb = [qpool.tile([128, s], bf16, name=f"kT{mt}", tag=f"kT{mt}")
                     for mt in range(MC)]
            # v, padded per head to 128 columns (ones col at A, zeros beyond)
            # so the AV lhsT is 128-wide and Fast Weight Load engages
            VP = 128
            v_sb = qpool.tile([128, NS, HPG, VP], bf16, name="v_sb", tag="v")
            nc.vector.memset(v_sb[:, :, :, A:VP], 0.0)
            vones_f = cpool.tile([128, NS * HPG], f32, name="vones_f")
            nc.vector.memset(vones_f[:], 1.0)
            nc.vector.tensor_copy(
                v_sb[:, :, :, A],
                vones_f[:].rearrange("p (t h) -> p t h", h=HPG))

            wo_sb = [wopool.tile([128, d], bf16, name=f"wo{kt}", tag=f"wo{kt}")
                     for kt in range(MC)]
            attn_sb = [qpool.tile([128, s], bf16, name=f"attn{t}", tag=f"at{t}")
                       for t in range(MC)]

            # ---------------- input loads ----------------
            # one big DMA per tensor: per-dma_start issue overhead (~1.2µs,
            # blocking on the issuing engine) dominates, so fewer is faster.
            # xT rides the scalar queue — it completes long before the first
            # exp needs ACT.
            xT_sb = xpool.tile([128, KD, s], bf16, name="xT_sb", tag="xT")
            w_sb = {
                wname: xpool.tile([128, KD, C], bf16, name=f"w{wname}",
                                  tag=f"w{wname}")
                for wname in ("q", "k", "v")
            }
            wds = {"q": wq_d, "k": wk_d, "v": wv_d}
            nc.sync.dma_start(
                w_sb["k"][:], wk_d[:].rearrange("(kt p) c -> p kt c", p=128))
            nc.scalar.dma_start(
                xT_sb[:, 0:KD // 2, :],
                xT_d[0:d // 2].rearrange("(kt p) t -> p kt t", p=128))
            nc.gpsimd.dma_start(
                xT_sb[:, KD // 2:, :],
                xT_d[d // 2:].rearrange("(kt p) t -> p kt t", p=128))
            nc.sync.dma_start(
                w_sb["q"][:], wq_d[:].rearrange("(kt p) c -> p kt c", p=128))
            nc.sync.dma_start(
                w_sb["v"][:], wv_d[:].rearrange("(kt p) c -> p kt c", p=128))
            nc.sync.dma_start(bq_sb[:], bqs_d[:, :])
            nc.sync.dma_start(bk_sb[:], bks_d[:, :])
            nc.sync.dma_start(bvb_sb[:], bvb_d[:, :])
            for kt in range(MC):
                nc.sync.dma_start(wo_sb[kt][:], wo_d[kt * 128:(kt + 1) * 128, :])
            nc.sync.dma_start(bob_sb[:], bob_d[:, :])

            # PE warm-up: the HAM clock-gate starts at 1.2GHz and needs ~4µs
            # of sustained busy to reach 2.4GHz. Dummy matmuls during the
            # input-DMA dead time warm it so the prologue runs at full clock.
            wrm = cpool.tile([128, QW], bf16, name="wrm")
            nc.vector.memset(wrm[:], 0.25)
            for _ in range(26):
                pw = pp1.tile([128, QW], f32, name="pw", tag="ps_qk")
                nc.tensor.matmul(pw[:], lhsT=wrm[:, 0:128], rhs=wrm[:],
                                 start=True, stop=True)

            # ---------------- building blocks ----------------
            def proj_qk_group(wname, mt, qc):
                # qT/kT[c, qs] = sum_d W[d, c] * xT[d, qs], bias via DVE
                qs = slice(qc * QW, (qc + 1) * QW)
                ps = pp1.tile([128, QW], f32, name="ps_qk", tag="ps_qk")
                for kt in range(KD):
                    nc.tensor.matmul(
                        ps[:],
                        lhsT=w_sb[wname][:, kt, mt * 128:(mt + 1) * 128],
                        rhs=xT_sb[:, kt, qs],
                        start=(kt == 0), stop=(kt == KD - 1),
                    )
                if wname == "k":
                    nc.vector.tensor_scalar_add(
                        kT_sb[mt][:, qs], ps[:], bk_sb[:, mt:mt + 1])
                else:
                    for hh in range(2):
                        rs = slice(hh * A, (hh + 1) * A)
                        nc.vector.tensor_scalar_add(
                            qTz_sb[mt][hh][rs, qs], ps[rs, :],
                            bq_sb[rs, mt:mt + 1])

            def proj_v_group(st):
                # v[s_tile, c] = sum_d xT[d, s_tile] * Wv[d, c]
                psv = pp1.tile([128, C], f32, name="psv", tag="ps_qk")
                for kt in range(KD):
                    nc.tensor.matmul(
                        psv[:],
                        lhsT=xT_sb[:, kt, st * 128:(st + 1) * 128],
                        rhs=w_sb["v"][:, kt, :],
                        start=(kt == 0), stop=(kt == KD - 1),
                    )
                nc.vector.tensor_add(
                    v_sb[:, st, :, 0:A],
                    psv[:].rearrange("p (h a) -> p h a", a=A),
                    bvb_sb[:].rearrange("p (h a) -> p h a", a=A),
                )

            def scores_group(p, qc, ng):
                # S^T for heads 2p (rows 0:64) / 2p+1 (rows 64:128); the two
                # K=64 matmuls hit disjoint PE row groups and co-run.
                # exp issued immediately after each head's scores.
                qs = slice(qc * QW, (qc + 1) * QW)
                sts = [stp.tile([128, KT_PER_ST * QW], f32,
                                name=f"st{hh}", tag=f"st{hh}")
                       for hh in range(2)]
                # full-K contraction against zero-padded qT keeps the PE in
                # uniform 128x128 mode (zeros annihilate the other head)
                for jj in range(KT_PER_ST):
                    kt = ng * KT_PER_ST + jj
                    for hh in range(2):
                        nc.tensor.matmul(
                            sts[hh][:, jj * QW:(jj + 1) * QW],
                            lhsT=kT_sb[p][:, kt * 128:(kt + 1) * 128],
                            rhs=qTz_sb[p][hh][:, qs],
                            start=True, stop=True,
                        )
                pts = []
                for hh in range(2):
                    pt = ptpool.tile([128, KT_PER_ST * QW], bf16,
                                     name=f"pt{hh}", tag=f"pt{hh}")
                    nc.scalar.activation(pt[:], sts[hh][:], AF.Exp,
                                         scale=0.125)
                    pts.append(pt)
                return pts

            def av_group(p, ng, avs, pts):
                heads = (2 * p, 2 * p + 1)
                for jj in range(KT_PER_ST):
                    kt = ng * KT_PER_ST + jj
                    for hh in range(2):
                        nc.tensor.matmul(
                            avs[hh][:],
                            lhsT=v_sb[:, kt, heads[hh], :],
                            rhs=pts[hh][:, jj * QW:(jj + 1) * QW],
                            start=(kt == 0), stop=(kt == NS - 1),
                        )

            def norm_unit(p, qc, avs):
                # attn[a, q] = av[a, q] * (1 / av[A, q]); recip on DVE,
                # column-broadcast on GpSimd, one DVE multiply.
                qs = slice(qc * QW, (qc + 1) * QW)
                for hh in range(2):
                    av = avs[hh]
                    off = hh * A
                    # custom-DVE recip must read SBUF (PSUM source gave
                    # garbage on HW) — copy the denominator row out first
                    den = spool.tile([1, QW], f32, name="den", tag="den")
                    nc.vector.tensor_copy(den[:], av[A:A + 1, :])
                    rec = spool.tile([1, QW], f32, name="rec", tag="rec")
                    nc.vector.reciprocal_approx_fast(rec[:], den[:])
                    bcd = spool.tile([A, QW], f32, name="bcd", tag="bcd")
                    nc.gpsimd.partition_broadcast(bcd[:], rec[:], channels=A)
                    nc.vector.tensor_mul(
                        attn_sb[p][off:off + A, qs], av[0:A, :], bcd[:])

            def fc_half(mt, nn, ob, eng=None):
                # half of out rows [mt*128, (mt+1)*128); DMA with the 2nd half
                ns_ = slice(nn * OW, (nn + 1) * OW)
                ps = pp1.tile([128, OW], f32, name="ps_o", tag="ps_qk")
                for kt in range(MC):
                    nc.tensor.matmul(
                        ps[:],
                        lhsT=attn_sb[kt][:, mt * 128:(mt + 1) * 128],
                        rhs=wo_sb[kt][:, ns_],
                        start=(kt == 0), stop=(kt == MC - 1),
                    )
                nc.vector.tensor_add(ob[:, ns_], ps[:], bob_sb[:, ns_])
                if nn == d // OW - 1:
                    (eng or nc.sync).dma_start(
                        out_d[mt * 128:(mt + 1) * 128, :], ob[:])

            def push_fc(mt, eng=None):
                ob = opool.tile([128, d], bf16, name="ob", tag="ob")
                for nn in range(d // OW - 1, -1, -1):
                    filler.appendleft(
                        (("fc", mt, nn),
                         (lambda mt=mt, nn=nn, ob=ob, eng=eng:
                          fc_half(mt, nn, ob, eng))))

            # ---------------- pipelined schedule ----------------
            # filler: PE work drained into slack inside ACT-bound stretches
            filler = deque()
            done = set()

            def push(key, fn):
                filler.append((key, fn))

            def drain(n=1):
                for _ in range(n):
                    if not filler:
                        return
                    key, fn = filler.popleft()
                    fn()
                    done.add(key)

            def drain_until(key):
                while key not in done and filler:
                    k, fn = filler.popleft()
                    fn()
                    done.add(k)

            for st in range(16):
                push(("v", st), (lambda st=st: proj_v_group(st)))
            for qc in range(1, QC):
                push(("qT", 0, qc), (lambda qc=qc: proj_qk_group("q", 0, qc)))
            for qc in range(QC):
                push(("kT", 1, qc), (lambda qc=qc: proj_qk_group("k", 1, qc)))
            for qc in range(QC):
                push(("qT", 1, qc), (lambda qc=qc: proj_qk_group("q", 1, qc)))

            units = [(0, 0), (0, 1), (0, 2), (1, 0), (0, 3), (1, 1),
                     (1, 2), (1, 3)]
            # fc chunk qc becomes ready once (1, qc) is normalized
            fc_ready_after = {(1, qc): qc for qc in range(QC)}

            # prologue: kT(0) chunk 0, qT(0) chunk 0
            proj_qk_group("k", 0, 0)
            done.add(("kT", 0, 0))
            proj_qk_group("q", 0, 0)
            done.add(("qT", 0, 0))

            for i, (p, qc) in enumerate(units):
                # prereqs of this unit's scores
                if p == 1:
                    drain_until(("kT", 1, QC - 1))
                    drain_until(("qT", 1, qc))
                else:
                    drain_until(("qT", 0, qc))
                avs = [avp.tile([128, QW], f32, name=f"av{hh}",
                                tag=f"av{hh}") for hh in range(2)]
                pts_list = []
                # AV lags its own scores by one ng: av(ng-1) runs right as
                # exp(ng-1) completes, so ACT never waits across phases
                for ng in range(NG):
                    need_kt = ("kT", p, min(QC - 1, (ng * KT_PER_ST + 1) // 4))
                    if p == 0 and need_kt not in done and i == 0:
                        proj_qk_group("k", 0, need_kt[2])
                        done.add(need_kt)
                    pts_list.append(scores_group(p, qc, ng))
                    if i == 0:
                        drain_until(("v", 2 * ng + 1))
                    else:
                        drain(1)
                    if ng > 0:
                        av_group(p, ng - 1, avs, pts_list[ng - 1])
                av_group(p, NG - 1, avs, pts_list[NG - 1])
                norm_unit(p, qc, avs)
                if (p, qc) in fc_ready_after:
                    fqc = fc_ready_after[(p, qc)]
                    last = fqc == QC - 1
                    for j, mt in enumerate(range(4 * fqc + 3, 4 * fqc - 1, -1)):
                        # tail chunk: spread final out-DMAs across engines
                        # (ACT is done by then) to flush the last writes fast
                        eng = ([nc.sync, nc.scalar, nc.gpsimd, nc.sync][j]
                               if last else None)
                        push_fc(mt, eng)

            # tail: leftover filler (last fc chunk)
            while filler:
                drain(1)

    nc.compile()
    return nc


def make_in_maps(x, Wq, bq, Wk, bk, Wv, bv, Wo, bo, n_cores=N_CORES):
    import ml_dtypes
    cf = ml_dtypes.bfloat16
    d = x.shape[2]
    MC = C // 128
    f = np.float32
    in_maps = []
    for core in range(n_cores):
        b, g = divmod(core, GROUPS)
        cs = slice(g * C, (g + 1) * C)
        bob = np.broadcast_to(bo, (128, d)).astype(f) if g == 0 else \
            np.zeros((128, d), f)
        m = {
            "xT": np.ascontiguousarray(x[b].T.astype(cf)),
            "wq": np.ascontiguousarray(Wq[:, cs].astype(cf)),
            "wk": np.ascontiguousarray(Wk[:, cs].astype(cf)),
            "wv": np.ascontiguousarray(Wv[:, cs].astype(cf)),
            "wo": np.ascontiguousarray(Wo[cs].astype(cf)),
            "bqs": np.ascontiguousarray(bq[cs].reshape(MC, 128).T, dtype=f),
            "bks": np.ascontiguousarray(bk[cs].reshape(MC, 128).T, dtype=f),
            "bvb": np.ascontiguousarray(np.broadcast_to(bv[cs], (128, C)), dtype=f),
            "bob": np.ascontiguousarray(bob),
        }
        in_maps.append(m)
    return in_maps


_nc_cache = {}


def _get_nc():
    if "nc" not in _nc_cache:
        _nc_cache["nc"] = build_nc()
    return _nc_cache["nc"]


def assemble(results):
    out = np.empty((B, S, D), np.float32)
    for b in range(B):
        acc = np.asarray(results[b * GROUPS]["out"], dtype=np.float32)
        for g in range(1, GROUPS):
            acc += np.asarray(results[b * GROUPS + g]["out"],
                              dtype=np.float32)
        out[b] = acc
    return out


def kernel(x, Wq, bq, Wk, bk, Wv, bv, Wo, bo, _trace=False, _mode=None):
    from concourse.bass_utils import run_bass_kernel_spmd

    nc = _get_nc()
    in_maps = make_in_maps(x, Wq, bq, Wk, bk, Wv, bv, Wo, bo)
    res = run_bass_kernel_spmd(nc, in_maps, core_ids=list(range(N_CORES)),
                               trace=_trace)
    _nc_cache["last_result"] = res
    return assemble(res.results)
